# revision 44
# baseline (speedup 1.0000x reference)
"""Exp-kernel multivariate Hawkes process log-likelihood on 8 Trainium2 cores.

Data-parallel: one sequence (length L=2048) per core. Per core the O(L^2)
pairwise exp-decay sum
    W[i,j] = ln(alpha*beta)[e_i,e_j] - beta[e_i,e_j] * (t_i - t_j)
is computed with one-hot factorized matmuls (host builds U (L,20) and
V (L,20); W = U @ V^T). All matmuls run in bf16 with a compensated hi/lo
split: x = x_hi + x_lo, the K=20 contraction becomes K=60 stacked as
hi*hi + hi*lo + lo*hi (dropped lo*lo ~ 2^-18 relative) - 1 PE col/cycle
with fast weight loads vs 4 cycles/col for fp32, at near-fp32 accuracy.
Per 128-row i-block: unmasked "prefix" columns (all j < block start) in
512-wide PSUM-bank pieces, plus a 128-wide diagonal band whose times are
recentered per block (i-block == j-block on the band, so offsets cancel
exactly) to shrink bf16 rounding of the dominant undecayed terms; a
strict-triangle -1e9 mask is added on the vector engine before exp.
exp() runs on the scalar engine with fused row-sum accumulation for
prefixes; band exps batch 4 blocks/op into an SBUF tile reduced by 4
vector ops. The O(L*D) compensator exp(ln(alpha) - beta*(T - t_i)) runs
on-device; linear terms (mu*T, colsum(alpha) gather) fold into a host
constant added after the device returns per-partition column sums.

Engine programs and semaphores are hand-written (raw bacc, no Tile
context) to avoid the Tile entry/exit barrier overhead.
"""
import numpy as np

B, L, D = 8, 2048, 10
NB = L // 128            # 16 i-blocks of 128 rows
CH = 512                 # PSUM bank width in fp32
NCORES = 8

_CACHE = {}


def _build_nc():
    import concourse.bacc as bacc
    from concourse import mybir

    f32 = mybir.dt.float32
    bf16 = mybir.dt.bfloat16
    Alu = mybir.AluOpType
    Act = mybir.ActivationFunctionType

    nc = bacc.Bacc()
    UV = nc.declare_dram_parameter("uv", [3 * 2 * D, 4 * L], bf16, isOutput=False)
    HTW = NB + D * NB + 128
    HT = nc.declare_dram_parameter("hostt", [128, HTW], f32, isOutput=False)
    OUT = nc.declare_dram_parameter("out", [128, 1], f32, isOutput=True)

    nmm = [(128 * ib + CH - 1) // CH + 1 for ib in range(NB)]  # prefix chunks + band
    cum_mm = np.cumsum(nmm).tolist()

    from contextlib import ExitStack
    with ExitStack() as ctx:
        uv = ctx.enter_context(nc.sbuf_tensor([3 * 2 * D, 4 * L], bf16))
        ht = ctx.enter_context(nc.sbuf_tensor([128, HTW], f32))
        mask = ctx.enter_context(nc.sbuf_tensor([128, 128], f32))
        wband = ctx.enter_context(nc.sbuf_tensor([128, NB * 128], f32))
        expb = ctx.enter_context(nc.sbuf_tensor([128, NB * 128], f32))
        exsink = ctx.enter_context(nc.sbuf_tensor([128, 15 * 128], f32))
        exc = ctx.enter_context(nc.sbuf_tensor([128, D * NB], f32))
        pfull = ctx.enter_context(nc.sbuf_tensor([128, NB], f32))
        pdiag = ctx.enter_context(nc.sbuf_tensor([128, NB], f32))
        inter = ctx.enter_context(nc.sbuf_tensor([128, NB], f32))
        lam = ctx.enter_context(nc.sbuf_tensor([128, NB], f32))
        acc = ctx.enter_context(nc.sbuf_tensor([128, NB + 1], f32))
        colsum = ctx.enter_context(nc.sbuf_tensor([128, 1], f32))
        slotA = ctx.enter_context(nc.psum_tensor([128, 4 * CH], f32))
        slotB = ctx.enter_context(nc.psum_tensor([128, 4 * CH], f32))
        uv_sem = ctx.enter_context(nc.semaphore("uv_sem"))
        ht_sem = ctx.enter_context(nc.semaphore("ht_sem"))
        pe_sem = ctx.enter_context(nc.semaphore("pe_sem"))
        dve_sem = ctx.enter_context(nc.semaphore("dve_sem"))
        act_sem = ctx.enter_context(nc.semaphore("act_sem"))
        out_sem = ctx.enter_context(nc.semaphore("out_sem"))
        block = ctx.enter_context(nc.Block())
        uv, ht, mask, wband, expb, exsink, exc = (
            uv[:], ht[:], mask[:], wband[:], expb[:], exsink[:], exc[:])
        pfull, pdiag, inter, lam, acc, colsum = (
            pfull[:], pdiag[:], inter[:], lam[:], acc[:], colsum[:])
        slotA, slotB = slotA[:], slotB[:]
        slots = [slotA, slotB]
        mug = ht[:, 0:NB]
        argc = ht[:, NB:NB + D * NB]
        iot = ht[:, NB + D * NB:NB + D * NB + 128]

        # ---- static schedules / semaphore value bookkeeping ----
        # DVE order: mask_build, mask_add x16, reduce x4, inter, lam, colsum
        dve_after_mask_add = [2 + ib for ib in range(NB)]     # mask_build=1
        dve_after_reduce4 = dve_after_mask_add[-1] + 4        # 21
        dve_after_inter = dve_after_reduce4 + 1               # 22
        dve_after_lam = dve_after_inter + 1                   # 23
        dve_after_colsum = dve_after_lam + 1                  # 24
        # ACT order: comp_exp, pfull0_zero, then per ib: prefix_exp (ib>=1),
        # band_exp after ib%4==3, finally Ln
        act_idx = 2
        act_after_prefix = {}
        act_after_band = {}
        for ib in range(NB):
            if ib >= 1:
                act_idx += 1
                act_after_prefix[ib] = act_idx
            if ib % 4 == 3:
                act_idx += 1
                act_after_band[ib // 4] = act_idx
        act_after_ln = act_idx + 1

        @block.sync
        def _(sync):
            sync.dma_start(out=uv[:, 0:2 * L], in_=UV[:, 0:2 * L]).then_inc(uv_sem, 16)
            sync.dma_start(out=uv[:, 2 * L:4 * L], in_=UV[:, 2 * L:4 * L]).then_inc(uv_sem, 16)
            sync.dma_start(out=ht[:], in_=HT[:]).then_inc(ht_sem, 16)
            sync.wait_ge(dve_sem, dve_after_colsum)
            sync.dma_start(out=OUT[:], in_=colsum[:]).then_inc(out_sem, 16)
            sync.wait_ge(out_sem, 16)

        @block.tensor
        def _(pe):
            pe.wait_ge(uv_sem, 32)
            for ib in range(NB):
                if ib >= 2:
                    # slot ib%2 free once block ib-2's consumers finished
                    pe.wait_ge(dve_sem, dve_after_mask_add[ib - 2])
                    if ib - 2 >= 1:
                        pe.wait_ge(act_sem, act_after_prefix[ib - 2])
                prow = slots[ib % 2]
                pw = 128 * ib
                for k in range(0, pw, CH):
                    n = min(CH, pw - k)
                    nc.tensor.matmul(
                        prow[:, k:k + n],
                        uv[:, ib * 128:(ib + 1) * 128],
                        uv[:, L + k:L + k + n],
                        start=True, stop=True,
                    ).then_inc(pe_sem, 1)
                nc.tensor.matmul(
                    prow[:, pw:pw + 128],
                    uv[:, 2 * L + ib * 128:2 * L + (ib + 1) * 128],
                    uv[:, 3 * L + pw:3 * L + pw + 128],
                    start=True, stop=True,
                ).then_inc(pe_sem, 1)

        @block.vector
        def _(dve):
            dve.wait_ge(ht_sem, 16)
            nc.vector.tensor_scalar(
                out=mask, in0=iot, scalar1=0.0, scalar2=-1e9,
                op0=Alu.is_ge, op1=Alu.mult,
            ).then_inc(dve_sem, 1)
            dve.wait_ge(dve_sem, 1)  # same-engine W->R on mask
            for ib in range(NB):
                pw = 128 * ib
                dve.wait_ge(pe_sem, cum_mm[ib])
                nc.vector.tensor_tensor(
                    out=wband[:, ib * 128:(ib + 1) * 128],
                    in0=slots[ib % 2][:, pw:pw + 128], in1=mask, op=Alu.add,
                ).then_inc(dve_sem, 1)
            for g in range(4):
                dve.wait_ge(act_sem, act_after_band[g])
                nc.vector.tensor_reduce(
                    out=pdiag[:, 4 * g:4 * (g + 1)],
                    in_=expb[:, g * 512:(g + 1) * 512].rearrange(
                        "p (b x) -> p b x", b=4),
                    axis=mybir.AxisListType.X, op=Alu.add,
                ).then_inc(dve_sem, 1)
            dve.wait_ge(act_sem, act_after_prefix[NB - 1])  # all pfull columns
            dve.wait_ge(dve_sem, dve_after_reduce4)         # own pdiag writes
            nc.vector.tensor_tensor(out=inter, in0=pfull, in1=pdiag,
                                    op=Alu.add).then_inc(dve_sem, 1)
            dve.wait_ge(dve_sem, dve_after_inter)
            nc.vector.tensor_tensor(out=lam, in0=inter, in1=mug,
                                    op=Alu.add).then_inc(dve_sem, 1)
            dve.wait_ge(act_sem, act_after_ln)
            nc.vector.tensor_reduce(
                out=colsum, in_=acc, axis=mybir.AxisListType.X, op=Alu.add,
            ).then_inc(dve_sem, 1)

        @block.scalar
        def _(act):
            act.wait_ge(ht_sem, 16)
            nc.scalar.activation(
                out=exc, in_=argc, func=Act.Exp, accum_out=acc[:, NB:NB + 1],
            ).then_inc(act_sem, 1)
            nc.scalar.activation(
                out=pfull[:, 0:1], in_=ht[:, 0:1], func=Act.Copy,
                bias=0.0, scale=0.0,
            ).then_inc(act_sem, 1)
            for ib in range(NB):
                pw = 128 * ib
                if ib >= 1:
                    act.wait_ge(pe_sem, cum_mm[ib])
                    if ib >= 2:
                        # same-engine WAW on exsink: wait own prior write
                        act.wait_ge(act_sem, act_after_prefix[ib - 1])
                    nc.scalar.activation(
                        out=exsink[:, :pw], in_=slots[ib % 2][:, :pw],
                        func=Act.Exp, accum_out=pfull[:, ib:ib + 1],
                    ).then_inc(act_sem, 1)
                if ib % 4 == 3:
                    g = ib // 4
                    act.wait_ge(dve_sem, dve_after_mask_add[ib])
                    nc.scalar.activation(
                        out=expb[:, g * 512:(g + 1) * 512],
                        in_=wband[:, g * 512:(g + 1) * 512], func=Act.Exp,
                    ).then_inc(act_sem, 1)
            act.wait_ge(dve_sem, dve_after_lam)
            nc.scalar.activation(out=acc[:, 0:NB], in_=lam,
                                 func=Act.Ln).then_inc(act_sem, 1)

    nc.finalize()
    return nc



def _softplus(x):
    return np.logaddexp(0.0, x.astype(np.float64))


def _bf16(x):
    import ml_dtypes
    return np.asarray(x).astype(ml_dtypes.bfloat16)


def _hilo_stacks(U, V):
    """K=60 bf16 compensated stacks: S = [U_hi;U_hi;U_lo], M = [V_hi;V_lo;V_hi].
    Returns (S^T, M^T) as (60, L) float arrays in bf16 values."""
    Uh = _bf16(U)
    Ul = _bf16(U - Uh.astype(np.float32))
    Vh = _bf16(V)
    Vl = _bf16(V - Vh.astype(np.float32))
    S = np.concatenate([Uh, Uh, Ul], axis=1)   # (L, 60)
    M = np.concatenate([Vh, Vl, Vh], axis=1)   # (L, 60)
    return S.T.copy(), M.T.copy()


def _host_prep(time_points, T, mu_raw, log_alpha, log_beta, event_types):
    """Per-core input tiles + additive host constants."""
    mu = _softplus(mu_raw).astype(np.float32)
    alpha = _softplus(log_alpha).astype(np.float32)
    beta = _softplus(log_beta).astype(np.float32)
    lnab = np.log(alpha.astype(np.float64) * beta.astype(np.float64)).astype(np.float32)
    colsumA = alpha.sum(0, dtype=np.float64)  # (D,)
    lna = np.log(alpha.astype(np.float64)).astype(np.float32)  # (D, D)
    iot = (np.arange(128, dtype=np.float32)[None, :]
           - np.arange(128, dtype=np.float32)[:, None])        # (128,128) x - p

    in_maps, consts = [], []
    for b in range(B):
        t = np.asarray(time_points[b], np.float32)
        e = np.asarray(event_types[b], np.int64)
        Tb = np.float64(T[b])

        U = np.empty((L, 2 * D), np.float32)
        U[:, :D] = lnab[e, :] - beta[e, :] * t[:, None]
        U[:, D:] = beta[e, :]
        E1 = np.zeros((L, D), np.float32)
        E1[np.arange(L), e] = 1.0
        V = np.concatenate([E1, E1 * t[:, None]], axis=1)
        # band variants: recenter t by each block's mean time (band matmuls
        # pair i and j from the same 128-block, so offsets cancel exactly)
        cblk = t.reshape(NB, 128).mean(axis=1).astype(np.float32)
        tb = (t - np.repeat(cblk, 128)).astype(np.float32)
        Ub = np.empty((L, 2 * D), np.float32)
        Ub[:, :D] = lnab[e, :] - beta[e, :] * tb[:, None]
        Ub[:, D:] = beta[e, :]
        Vb = np.concatenate([E1, E1 * tb[:, None]], axis=1)
        Sp, Mp = _hilo_stacks(U, V)
        Sb, Mb = _hilo_stacks(Ub, Vb)

        # (p, ib) layout: i = 128*ib + p
        mug = mu[e].reshape(NB, 128).T.copy()                      # (128, NB)
        dt = (np.float32(Tb) - t).astype(np.float32)
        argc = (lna[:, e] - beta[:, e] * dt[None, :]).astype(np.float32)  # (D, L)
        # (128, D*NB): col d*NB + ib <-> i = 128*ib + p
        argc = argc.reshape(D, NB, 128).transpose(2, 0, 1).reshape(128, D * NB).copy()

        const = -Tb * mu.sum(dtype=np.float64) - colsumA[e].sum()
        uv = _bf16(np.concatenate([Sp, Mp, Sb, Mb], axis=1))  # (60, 4L) bf16
        ht = np.concatenate([mug, argc, iot], axis=1)      # (128, NB+D*NB+128)
        in_maps.append({
            "uv": np.ascontiguousarray(uv),
            "hostt": np.ascontiguousarray(ht),
        })
        consts.append(np.float32(const))
    return in_maps, consts


def kernel(**inputs):
    from concourse.bass_utils import run_bass_kernel_spmd

    if "nc" not in _CACHE:
        _CACHE["nc"] = _build_nc()
    nc = _CACHE["nc"]

    in_maps, consts = _host_prep(**inputs)
    res = run_bass_kernel_spmd(nc, in_maps, list(range(NCORES)))
    out = np.empty(B, np.float32)
    for b in range(B):
        cs = res.results[b]["out"].reshape(128)
        out[b] = np.float32(np.float32(cs.sum(dtype=np.float32)) + consts[b])
    return out


# revision 45
# speedup vs baseline: 1.0234x; 1.0234x over previous
"""Exp-kernel multivariate Hawkes process log-likelihood on 8 Trainium2 cores.

Data-parallel: one sequence (length L=2048) per core. Per core the O(L^2)
pairwise exp-decay sum
    W[i,j] = ln(alpha*beta)[e_i,e_j] - beta[e_i,e_j] * (t_i - t_j)
is computed with one-hot factorized matmuls (host builds U (L,20) and
V (L,20); W = U @ V^T). All matmuls run in bf16 with a compensated hi/lo
split: x = x_hi + x_lo, the K=20 contraction becomes K=60 stacked as
hi*hi + hi*lo + lo*hi (dropped lo*lo ~ 2^-18 relative) - 1 PE col/cycle
with fast weight loads vs 4 cycles/col for fp32, at near-fp32 accuracy.
Per 128-row i-block: unmasked "prefix" columns (all j < block start) in
512-wide PSUM-bank pieces, plus a 128-wide diagonal band whose times are
recentered per block (i-block == j-block on the band, so offsets cancel
exactly) to shrink bf16 rounding of the dominant undecayed terms; a
strict-triangle -1e9 mask is added on the vector engine before exp.
exp() runs on the scalar engine with fused row-sum accumulation for
prefixes; band exps batch 4 blocks/op into an SBUF tile reduced by 4
vector ops. The O(L*D) compensator exp(ln(alpha) - beta*(T - t_i)) runs
on-device; linear terms (mu*T, colsum(alpha) gather) fold into a host
constant added after the device returns per-partition column sums.

Engine programs and semaphores are hand-written (raw bacc, no Tile
context) to avoid the Tile entry/exit barrier overhead.
"""
import numpy as np

B, L, D = 8, 2048, 10
NB = L // 128            # 16 i-blocks of 128 rows
CH = 512                 # PSUM bank width in fp32
NCORES = 8

_CACHE = {}


def _build_nc():
    import concourse.bacc as bacc
    from concourse import mybir

    f32 = mybir.dt.float32
    bf16 = mybir.dt.bfloat16
    Alu = mybir.AluOpType
    Act = mybir.ActivationFunctionType

    nc = bacc.Bacc()
    UV = nc.declare_dram_parameter("uv", [3 * 2 * D, 4 * L], bf16, isOutput=False)
    HTW = NB + D * NB + 128
    HT = nc.declare_dram_parameter("hostt", [128, HTW], f32, isOutput=False)
    OUT = nc.declare_dram_parameter("out", [128, 1], f32, isOutput=True)

    nmm = [(128 * ib + CH - 1) // CH + 1 for ib in range(NB)]  # prefix chunks + band
    cum_mm = np.cumsum(nmm).tolist()

    from contextlib import ExitStack
    with ExitStack() as ctx:
        uv = ctx.enter_context(nc.sbuf_tensor([3 * 2 * D, 4 * L], bf16))
        ht = ctx.enter_context(nc.sbuf_tensor([128, HTW], f32))
        mask = ctx.enter_context(nc.sbuf_tensor([128, 128], f32))
        wband = ctx.enter_context(nc.sbuf_tensor([128, NB * 128], f32))
        expb = ctx.enter_context(nc.sbuf_tensor([128, NB * 128], f32))
        exsink = ctx.enter_context(nc.sbuf_tensor([128, 15 * 128], f32))
        exc = ctx.enter_context(nc.sbuf_tensor([128, D * NB], f32))
        pfull = ctx.enter_context(nc.sbuf_tensor([128, NB], f32))
        pdiag = ctx.enter_context(nc.sbuf_tensor([128, NB], f32))
        inter = ctx.enter_context(nc.sbuf_tensor([128, NB], f32))
        lam = ctx.enter_context(nc.sbuf_tensor([128, NB], f32))
        acc = ctx.enter_context(nc.sbuf_tensor([128, NB + 1], f32))
        colsum = ctx.enter_context(nc.sbuf_tensor([128, 1], f32))
        slotA = ctx.enter_context(nc.psum_tensor([128, 4 * CH], f32))
        slotB = ctx.enter_context(nc.psum_tensor([128, 4 * CH], f32))
        uv_sem = ctx.enter_context(nc.semaphore("uv_sem"))
        ht_sem = ctx.enter_context(nc.semaphore("ht_sem"))
        pe_sem = ctx.enter_context(nc.semaphore("pe_sem"))
        dve_sem = ctx.enter_context(nc.semaphore("dve_sem"))
        act_sem = ctx.enter_context(nc.semaphore("act_sem"))
        out_sem = ctx.enter_context(nc.semaphore("out_sem"))
        block = ctx.enter_context(nc.Block(no_gpsimd_drain=True))
        uv, ht, mask, wband, expb, exsink, exc = (
            uv[:], ht[:], mask[:], wband[:], expb[:], exsink[:], exc[:])
        pfull, pdiag, inter, lam, acc, colsum = (
            pfull[:], pdiag[:], inter[:], lam[:], acc[:], colsum[:])
        slotA, slotB = slotA[:], slotB[:]
        slots = [slotA, slotB]
        mug = ht[:, 0:NB]
        argc = ht[:, NB:NB + D * NB]
        iot = ht[:, NB + D * NB:NB + D * NB + 128]

        # ---- static schedules / semaphore value bookkeeping ----
        # DVE order: mask_build, mask_add x16, reduce x4, inter, lam, colsum
        dve_after_mask_add = [2 + ib for ib in range(NB)]     # mask_build=1
        dve_after_reduce4 = dve_after_mask_add[-1] + 4        # 21
        dve_after_inter = dve_after_reduce4 + 1               # 22
        dve_after_lam = dve_after_inter + 1                   # 23
        dve_after_colsum = dve_after_lam + 1                  # 24
        # ACT order: comp_exp, pfull0_zero, then per ib: prefix_exp (ib>=1),
        # band_exp after ib%4==3, finally Ln
        act_idx = 2
        act_after_prefix = {}
        act_after_band = {}
        for ib in range(NB):
            if ib >= 1:
                act_idx += 1
                act_after_prefix[ib] = act_idx
            if ib % 4 == 3:
                act_idx += 1
                act_after_band[ib // 4] = act_idx
        act_after_ln = act_idx + 1

        @block.sync
        def _(sync):
            sync.dma_start(out=uv[:, 0:2 * L], in_=UV[:, 0:2 * L]).then_inc(uv_sem, 16)
            sync.dma_start(out=uv[:, 2 * L:4 * L], in_=UV[:, 2 * L:4 * L]).then_inc(uv_sem, 16)
            sync.dma_start(out=ht[:], in_=HT[:]).then_inc(ht_sem, 16)
            sync.wait_ge(dve_sem, dve_after_colsum)
            sync.dma_start(out=OUT[:], in_=colsum[:]).then_inc(out_sem, 16)
            sync.wait_ge(out_sem, 16)

        @block.tensor
        def _(pe):
            pe.wait_ge(uv_sem, 32)
            for ib in range(NB):
                if ib >= 2:
                    # slot ib%2 free once block ib-2's consumers finished
                    pe.wait_ge(dve_sem, dve_after_mask_add[ib - 2])
                    if ib - 2 >= 1:
                        pe.wait_ge(act_sem, act_after_prefix[ib - 2])
                prow = slots[ib % 2]
                pw = 128 * ib
                for k in range(0, pw, CH):
                    n = min(CH, pw - k)
                    nc.tensor.matmul(
                        prow[:, k:k + n],
                        uv[:, ib * 128:(ib + 1) * 128],
                        uv[:, L + k:L + k + n],
                        start=True, stop=True,
                    ).then_inc(pe_sem, 1)
                nc.tensor.matmul(
                    prow[:, pw:pw + 128],
                    uv[:, 2 * L + ib * 128:2 * L + (ib + 1) * 128],
                    uv[:, 3 * L + pw:3 * L + pw + 128],
                    start=True, stop=True,
                ).then_inc(pe_sem, 1)

        @block.vector
        def _(dve):
            dve.wait_ge(ht_sem, 16)
            nc.vector.tensor_scalar(
                out=mask, in0=iot, scalar1=0.0, scalar2=-1e9,
                op0=Alu.is_ge, op1=Alu.mult,
            ).then_inc(dve_sem, 1)
            dve.wait_ge(dve_sem, 1)  # same-engine W->R on mask
            for ib in range(NB):
                pw = 128 * ib
                dve.wait_ge(pe_sem, cum_mm[ib])
                nc.vector.tensor_tensor(
                    out=wband[:, ib * 128:(ib + 1) * 128],
                    in0=slots[ib % 2][:, pw:pw + 128], in1=mask, op=Alu.add,
                ).then_inc(dve_sem, 1)
            for g in range(4):
                dve.wait_ge(act_sem, act_after_band[g])
                nc.vector.tensor_reduce(
                    out=pdiag[:, 4 * g:4 * (g + 1)],
                    in_=expb[:, g * 512:(g + 1) * 512].rearrange(
                        "p (b x) -> p b x", b=4),
                    axis=mybir.AxisListType.X, op=Alu.add,
                ).then_inc(dve_sem, 1)
            dve.wait_ge(act_sem, act_after_prefix[NB - 1])  # all pfull columns
            dve.wait_ge(dve_sem, dve_after_reduce4)         # own pdiag writes
            nc.vector.tensor_tensor(out=inter, in0=pfull, in1=pdiag,
                                    op=Alu.add).then_inc(dve_sem, 1)
            dve.wait_ge(dve_sem, dve_after_inter)
            nc.vector.tensor_tensor(out=lam, in0=inter, in1=mug,
                                    op=Alu.add).then_inc(dve_sem, 1)
            dve.wait_ge(act_sem, act_after_ln)
            nc.vector.tensor_reduce(
                out=colsum, in_=acc, axis=mybir.AxisListType.X, op=Alu.add,
            ).then_inc(dve_sem, 1)

        @block.scalar
        def _(act):
            act.wait_ge(ht_sem, 16)
            nc.scalar.activation(
                out=exc, in_=argc, func=Act.Exp, accum_out=acc[:, NB:NB + 1],
            ).then_inc(act_sem, 1)
            nc.scalar.activation(
                out=pfull[:, 0:1], in_=ht[:, 0:1], func=Act.Copy,
                bias=0.0, scale=0.0,
            ).then_inc(act_sem, 1)
            for ib in range(NB):
                pw = 128 * ib
                if ib >= 1:
                    act.wait_ge(pe_sem, cum_mm[ib])
                    if ib >= 2:
                        # same-engine WAW on exsink: wait own prior write
                        act.wait_ge(act_sem, act_after_prefix[ib - 1])
                    nc.scalar.activation(
                        out=exsink[:, :pw], in_=slots[ib % 2][:, :pw],
                        func=Act.Exp, accum_out=pfull[:, ib:ib + 1],
                    ).then_inc(act_sem, 1)
                if ib % 4 == 3:
                    g = ib // 4
                    act.wait_ge(dve_sem, dve_after_mask_add[ib])
                    nc.scalar.activation(
                        out=expb[:, g * 512:(g + 1) * 512],
                        in_=wband[:, g * 512:(g + 1) * 512], func=Act.Exp,
                    ).then_inc(act_sem, 1)
            act.wait_ge(dve_sem, dve_after_lam)
            nc.scalar.activation(out=acc[:, 0:NB], in_=lam,
                                 func=Act.Ln).then_inc(act_sem, 1)

    nc.finalize()
    return nc



def _softplus(x):
    return np.logaddexp(0.0, x.astype(np.float64))


def _bf16(x):
    import ml_dtypes
    return np.asarray(x).astype(ml_dtypes.bfloat16)


def _hilo_stacks(U, V):
    """K=60 bf16 compensated stacks: S = [U_hi;U_hi;U_lo], M = [V_hi;V_lo;V_hi].
    Returns (S^T, M^T) as (60, L) float arrays in bf16 values."""
    Uh = _bf16(U)
    Ul = _bf16(U - Uh.astype(np.float32))
    Vh = _bf16(V)
    Vl = _bf16(V - Vh.astype(np.float32))
    S = np.concatenate([Uh, Uh, Ul], axis=1)   # (L, 60)
    M = np.concatenate([Vh, Vl, Vh], axis=1)   # (L, 60)
    return S.T.copy(), M.T.copy()


def _host_prep(time_points, T, mu_raw, log_alpha, log_beta, event_types):
    """Per-core input tiles + additive host constants."""
    mu = _softplus(mu_raw).astype(np.float32)
    alpha = _softplus(log_alpha).astype(np.float32)
    beta = _softplus(log_beta).astype(np.float32)
    lnab = np.log(alpha.astype(np.float64) * beta.astype(np.float64)).astype(np.float32)
    colsumA = alpha.sum(0, dtype=np.float64)  # (D,)
    lna = np.log(alpha.astype(np.float64)).astype(np.float32)  # (D, D)
    iot = (np.arange(128, dtype=np.float32)[None, :]
           - np.arange(128, dtype=np.float32)[:, None])        # (128,128) x - p

    in_maps, consts = [], []
    for b in range(B):
        t = np.asarray(time_points[b], np.float32)
        e = np.asarray(event_types[b], np.int64)
        Tb = np.float64(T[b])

        U = np.empty((L, 2 * D), np.float32)
        U[:, :D] = lnab[e, :] - beta[e, :] * t[:, None]
        U[:, D:] = beta[e, :]
        E1 = np.zeros((L, D), np.float32)
        E1[np.arange(L), e] = 1.0
        V = np.concatenate([E1, E1 * t[:, None]], axis=1)
        # band variants: recenter t by each block's mean time (band matmuls
        # pair i and j from the same 128-block, so offsets cancel exactly)
        cblk = t.reshape(NB, 128).mean(axis=1).astype(np.float32)
        tb = (t - np.repeat(cblk, 128)).astype(np.float32)
        Ub = np.empty((L, 2 * D), np.float32)
        Ub[:, :D] = lnab[e, :] - beta[e, :] * tb[:, None]
        Ub[:, D:] = beta[e, :]
        Vb = np.concatenate([E1, E1 * tb[:, None]], axis=1)
        Sp, Mp = _hilo_stacks(U, V)
        Sb, Mb = _hilo_stacks(Ub, Vb)

        # (p, ib) layout: i = 128*ib + p
        mug = mu[e].reshape(NB, 128).T.copy()                      # (128, NB)
        dt = (np.float32(Tb) - t).astype(np.float32)
        argc = (lna[:, e] - beta[:, e] * dt[None, :]).astype(np.float32)  # (D, L)
        # (128, D*NB): col d*NB + ib <-> i = 128*ib + p
        argc = argc.reshape(D, NB, 128).transpose(2, 0, 1).reshape(128, D * NB).copy()

        const = -Tb * mu.sum(dtype=np.float64) - colsumA[e].sum()
        uv = _bf16(np.concatenate([Sp, Mp, Sb, Mb], axis=1))  # (60, 4L) bf16
        ht = np.concatenate([mug, argc, iot], axis=1)      # (128, NB+D*NB+128)
        in_maps.append({
            "uv": np.ascontiguousarray(uv),
            "hostt": np.ascontiguousarray(ht),
        })
        consts.append(np.float32(const))
    return in_maps, consts


def kernel(**inputs):
    from concourse.bass_utils import run_bass_kernel_spmd

    if "nc" not in _CACHE:
        _CACHE["nc"] = _build_nc()
    nc = _CACHE["nc"]

    in_maps, consts = _host_prep(**inputs)
    res = run_bass_kernel_spmd(nc, in_maps, list(range(NCORES)))
    out = np.empty(B, np.float32)
    for b in range(B):
        cs = res.results[b]["out"].reshape(128)
        out[b] = np.float32(np.float32(cs.sum(dtype=np.float32)) + consts[b])
    return out
